# revision 18
# baseline (speedup 1.0000x reference)
"""MeanStdMax pooling kernel for Trainium2 (8 NeuronCores, data-parallel).

Input : hidden_states [16, 13, 512, 768] fp32
Output: [16, 13, 2304] fp32 = concat([sum(seq), std(seq, ddof=1), max(seq)], -1)

Sharding: batch dim 16 -> 2 batches per core (no cross-core communication).

Per-core plan (26 (b,l) pairs, each [512, 768]):
  - DMA each pair as one [128, 4*768] tile; partition p holds seq rows
    4p..4p+3, so every partition is one contiguous 12KB DRAM chunk.  The 16
    DMA engines sustain ~355GB/s aggregate and input fully lands at ~124us;
    everything else is organized so (a) nothing with an unresolved dep ever
    sits in the sync queue in front of an input DMA, and (b) the tail after
    the last input byte is minimal.
  - sum  : fp32r one-hot-weight matmuls straight off the raw tile; PSUM row
           accumulates pair j's per-hidden sums (partition reduce on the PE).
           PSUM accumulation is split into TWO groups (pairs 0-19 / 20-25)
           so the big stats epilogue runs hidden mid-stream; only a 6-row
           epilogue remains after the final tile.  Its DMA issues after the
           loop so it can never block input prefetch.
  - sumsq: ACT Square -> bf16, then bf16 one-hot matmuls into 2nd accumulator.
  - max  : DVE max tree over the 4 seq blocks -> M [128,768] bf16; gpsimd
           partition_all_reduce -> fp32 Mred; row 0 DMA'd out per pair with
           a 5-pair lag (dep resolved ~12us before issue -> no queue stall).
  - last two tiles stream as two half-tiles each so the DVE tree and ACT
    square of the final pair start ~1.5us earlier.
  - epilogue: std = sqrt((sumsq - sum^2/512)/511).
"""

import os
import sys

import numpy as np

for _p in ("/opt/trn_rl_repo", "/root/.axon_site/_ro/trn_rl_repo"):
    if os.path.isdir(_p) and _p not in sys.path:
        sys.path.insert(0, _p)

import concourse.bacc as bacc
import concourse.bass as bass
import concourse.bass_isa as bass_isa
import concourse.mybir as mybir
import concourse.tile as tile
from concourse.bass_utils import run_bass_kernel_spmd

N_CORES = 8
B_FULL, L, S, H = 16, 13, 512, 768
B = B_FULL // N_CORES  # 2 batches per core
P = 128
NBLK = S // P  # 4
NPAIR = B * L  # 26
F32 = mybir.dt.float32
F32R = mybir.dt.float32r
BF16 = mybir.dt.bfloat16

GSPLIT = 20              # stats psum groups: pairs [0,20) and [20,26)
SPLIT_TILES = (24, 25)   # stream these pairs as two half-tiles
# gpsimd partition_all_reduce batching: (first pair, npairs).  2-pair calls
# halve the per-call overhead so gpsimd keeps up with the ~4.1us/pair input
# cadence; the last two pairs stay singletons so the tail chain after the
# final tile is one short reduce, overlapped with the previous one.
MAXG = [(0, 2), (2, 2), (4, 2), (6, 2), (8, 2), (10, 2), (12, 2), (14, 2),
        (16, 2), (18, 2), (20, 2), (22, 2), (24, 1), (25, 1)]

_CACHE = {}


def _build():
    if "nc" in _CACHE:
        return _CACHE["nc"]

    nc = bacc.Bacc("TRN2", target_bir_lowering=False, debug=False,
                   num_devices=N_CORES)
    # float32r: same bits as fp32, but satisfies the BIR verifier's
    # "rounded to FP32r" rule so DMA-loaded tiles can feed fp32r matmuls
    # (the fast single-pass fp32 PE mode, ~0.5ns/row vs 1.7 for fp32).
    x = nc.dram_tensor("x", [B, L, S, H], F32R, kind="ExternalInput").ap()
    out = nc.dram_tensor("out", [B, L, 3 * H], F32, kind="ExternalOutput").ap()
    out2 = out.rearrange("b l h -> (b l) h")  # [26, 2304]

    with tile.TileContext(nc) as tc:
        with (
            tc.tile_pool(name="inp", bufs=6) as in_pool,
            tc.tile_pool(name="sq", bufs=4) as sq_pool,
            tc.tile_pool(name="acc", bufs=4) as acc_pool,
            tc.tile_pool(name="mgrp", bufs=3) as mgrp_pool,
            tc.tile_pool(name="mred", bufs=6) as mred_pool,
            tc.tile_pool(name="const", bufs=1) as const_pool,
            tc.tile_pool(name="ep", bufs=1) as ep_pool,
            tc.tile_pool(name="psum", bufs=1, space="PSUM") as psum_pool,
        ):
            # one-hot weight bank: W[:, 26-j : 58-j] is all-ones exactly at
            # local column j.
            W0 = const_pool.tile([P, NPAIR + 32], F32)
            nc.gpsimd.memset(W0[:], 0.0)
            nc.gpsimd.memset(W0[:, NPAIR:NPAIR + 1], 1.0)
            Wr = const_pool.tile([P, NPAIR + 32], F32R)
            nc.vector.tensor_copy(Wr[:], W0[:])
            Wb = const_pool.tile([P, NPAIR + 32], BF16)
            nc.vector.tensor_copy(Wb[:], W0[:])

            # two psum accumulation groups (4 banks each)
            ps = []
            for g in range(2):
                ps.append({
                    "sum_a": psum_pool.tile([32, 512], F32, name=f"sum_a{g}",
                                            tag=f"sum_a{g}"),
                    "sum_b": psum_pool.tile([32, 256], F32, name=f"sum_b{g}",
                                            tag=f"sum_b{g}"),
                    "sq_a": psum_pool.tile([32, 512], F32, name=f"sq_a{g}",
                                           tag=f"sq_a{g}"),
                    "sq_b": psum_pool.tile([32, 256], F32, name=f"sq_b{g}",
                                           tag=f"sq_b{g}"),
                })

            def grp(j):
                return 0 if j < GSPLIT else 1

            def is_start(j):
                return j in (0, GSPLIT)

            def is_stop(j):
                return j in (GSPLIT - 1, NPAIR - 1)

            def local(j):
                return j - (0 if j < GSPLIT else GSPLIT)

            def emit_epilogue(g, lo, hi):
                # std = sqrt((sumsq - sum^2/n)/(n-1)); also stages sums.
                # psum rows are group-local; returns the stats tile, the DMA
                # is issued separately (after the loop; see module docstring).
                n = hi - lo
                stats = ep_pool.tile([n, 2 * H], F32, tag=f"stats{g}")
                nc.scalar.copy(stats[:, 0:512], ps[g]["sum_a"][0:n])
                nc.scalar.copy(stats[:, 512:768], ps[g]["sum_b"][0:n])
                # sum^2/n on ACT: Square(x/sqrt(n))
                sum2 = ep_pool.tile([n, H], F32, tag=f"sum2{g}")
                nc.scalar.activation(sum2[:], stats[:, 0:H],
                                     mybir.ActivationFunctionType.Square,
                                     scale=1.0 / float(np.sqrt(S)))
                var = ep_pool.tile([n, H], F32, tag=f"var{g}")
                nc.vector.tensor_tensor(var[:, 0:512], ps[g]["sq_a"][0:n],
                                        sum2[:, 0:512],
                                        op=mybir.AluOpType.subtract)
                nc.vector.tensor_tensor(var[:, 512:768], ps[g]["sq_b"][0:n],
                                        sum2[:, 512:768],
                                        op=mybir.AluOpType.subtract)
                nc.scalar.activation(stats[:, H:2 * H], var[:],
                                     mybir.ActivationFunctionType.Sqrt,
                                     scale=1.0 / (S - 1))
                return stats

            # PE runs one pair behind for sq matmuls so its per-iteration
            # work only depends on data from iteration j-1.
            pending = None  # (j, Q_tile)
            # max-out DMAs (one per reduce group) lag 4 groups: their gpsimd
            # Mred dep resolves well before they issue, so they never stall
            # the sync queue in front of input prefetch DMAs.
            max_outs = []

            def flush_max_outs(keep):
                while len(max_outs) > keep:
                    g0, glen, mred = max_outs.pop(0)
                    nc.sync.dma_start(out2[g0:g0 + glen, 2 * H:3 * H],
                                      mred[0:1, 0:glen * H])

            def emit_tail(j, Q):
                g = grp(j)
                first, last = is_start(j), is_stop(j)
                lj = local(j)
                wjb = Wb[:, NPAIR - lj:NPAIR - lj + 32]
                Qv = Q[:].rearrange("p (n h) -> p n h", h=H)
                for blk in range(NBLK):
                    nc.tensor.matmul(
                        ps[g]["sq_a"][:], wjb, Qv[:, blk, 0:512],
                        start=first and blk == 0, stop=last and blk == NBLK - 1)
                    nc.tensor.matmul(
                        ps[g]["sq_b"][:], wjb, Qv[:, blk, 512:768],
                        start=first and blk == 0, stop=last and blk == NBLK - 1)

            stats_a = None
            gi = 0            # index into MAXG
            Mgrp = Mred = None

            for j in range(NPAIR):
                b, l = divmod(j, L)
                g = grp(j)
                first, last = is_start(j), is_stop(j)
                g0, glen = MAXG[gi]
                if j == g0:  # new reduce group
                    Mgrp = mgrp_pool.tile([P, 2 * H], BF16, tag="Mgrp")
                    Mred = mred_pool.tile([P, 2 * H], F32, tag="Mred")
                k = j - g0

                T = in_pool.tile([P, NBLK * H], F32R)
                Tr = T[:].rearrange("p (n h) -> p n h", h=H)
                # partition p <- seq rows 4p..4p+3: contiguous 12KB chunks;
                # the seq->(p,i) mapping is irrelevant to sum/max/sumsq.
                src = x[b, l].rearrange("(p n) h -> p n h", n=NBLK)
                if j in SPLIT_TILES:
                    nc.sync.dma_start(Tr[:, 0:2, :], src[:, 0:2, :])
                    nc.sync.dma_start(Tr[:, 2:4, :], src[:, 2:4, :])
                else:
                    nc.sync.dma_start(T[:], src)
                Tv = T[:].bitcast(F32).rearrange("p (n h) -> p n h", h=H)

                # ---- sums: fp32r one-hot matmuls straight off the raw tile ----
                lj = local(j)
                wjr = Wr[:, NPAIR - lj:NPAIR - lj + 32]
                for blk in range(NBLK):
                    nc.tensor.matmul(
                        ps[g]["sum_a"][:], wjr, Tr[:, blk, 0:512],
                        start=first and blk == 0, stop=last and blk == NBLK - 1)
                    nc.tensor.matmul(
                        ps[g]["sum_b"][:], wjr, Tr[:, blk, 512:768],
                        start=first and blk == 0, stop=last and blk == NBLK - 1)

                # ---- max tree on DVE (final level bf16) ----
                m2 = acc_pool.tile([P, 2 * H], F32, tag="m2")
                m2v = m2[:].rearrange("p (n h) -> p n h", h=H)
                if j in SPLIT_TILES:
                    nc.vector.tensor_tensor(
                        m2v[:, 0, :], Tv[:, 0, :], Tv[:, 1, :],
                        op=mybir.AluOpType.max)
                    nc.vector.tensor_tensor(
                        m2v[:, 1, :], Tv[:, 2, :], Tv[:, 3, :],
                        op=mybir.AluOpType.max)
                else:
                    nc.vector.tensor_tensor(
                        m2v, Tv[:, 0:2, :], Tv[:, 2:4, :],
                        op=mybir.AluOpType.max)
                nc.vector.tensor_tensor(Mgrp[:, k * H:(k + 1) * H],
                                        m2v[:, 0, :], m2v[:, 1, :],
                                        op=mybir.AluOpType.max)

                if j == g0 + glen - 1:
                    # group's max columns complete: one partition all-reduce
                    # for the whole group on the (otherwise idle) gpsimd
                    nc.gpsimd.partition_all_reduce(
                        Mred[:, 0:glen * H], Mgrp[:, 0:glen * H],
                        channels=P, reduce_op=bass_isa.ReduceOp.max)
                    max_outs.append((g0, glen, Mred))
                    gi += 1

                # ---- squares in bf16 on ACT ----
                Q = sq_pool.tile([P, NBLK * H], BF16)
                if j in SPLIT_TILES:
                    nc.scalar.activation(Q[:, 0:2 * H],
                                         T[:, 0:2 * H].bitcast(F32),
                                         mybir.ActivationFunctionType.Square)
                    nc.scalar.activation(Q[:, 2 * H:4 * H],
                                         T[:, 2 * H:4 * H].bitcast(F32),
                                         mybir.ActivationFunctionType.Square)
                else:
                    nc.scalar.activation(Q[:], T[:].bitcast(F32),
                                         mybir.ActivationFunctionType.Square)

                if pending is not None:
                    emit_tail(*pending)
                pending = (j, Q)
                if j == GSPLIT:
                    # group A psum complete (its last sq matmuls just
                    # emitted); big epilogue compute runs hidden mid-stream.
                    stats_a = emit_epilogue(0, 0, GSPLIT)
                flush_max_outs(keep=4)

            emit_tail(*pending)
            # ---- tail: everything below is after all input DMAs ----
            nc.sync.dma_start(out2[0:GSPLIT, 0:2 * H], stats_a[:])
            flush_max_outs(keep=0)
            stats_b = emit_epilogue(1, GSPLIT, NPAIR)
            nc.sync.dma_start(out2[GSPLIT:NPAIR, 0:2 * H], stats_b[:])

    nc.compile()
    _CACHE["nc"] = nc
    return nc


def _run(hidden_states: np.ndarray, trace: bool = False):
    nc = _build()
    x = np.ascontiguousarray(np.asarray(hidden_states, dtype=np.float32))
    assert x.shape == (B_FULL, L, S, H), x.shape
    in_maps = [{"x": x[c * B:(c + 1) * B]} for c in range(N_CORES)]
    res = run_bass_kernel_spmd(nc, in_maps, core_ids=list(range(N_CORES)),
                               trace=trace)
    out = np.empty((B_FULL, L, 3 * H), dtype=np.float32)
    for c in range(N_CORES):
        out[c * B:(c + 1) * B] = res.results[c]["out"]
    return out, res


def kernel(hidden_states: np.ndarray) -> np.ndarray:
    out, _ = _run(hidden_states)
    return out
